# revision 1
# baseline (speedup 1.0000x reference)
"""Multi-head causal attention (B=4, L=2048, D=1024, H=16, dh=64) on 8 TRN2 NeuronCores.

Sharding: core i handles batch b = i//2 and head-group g = i%2 (8 heads each).
No cross-core communication needed: each core computes o[b, :, g*512:(g+1)*512].

Per-core dataflow (all layouts chosen so matmul contraction is on partitions):
  inputs (host-prepared, bf16, tiled so every DMA is a single contiguous ~1MB read):
    qTt/kTt/vTt [512, 4096]: row lb*128+p, col d*512+c  holds  x[b][lb*512+c, d*128+p]
    wq/wk/wv    [128, 4096]: row p,       col d*512+c  holds  W[d*128+p, c]
  projections (bf16 matmuls, fp32 psum):
    qwT/kwT [128(2 heads x 64dh), L] bf16;  vw_aug [128(Lk sub), 8*65] bf16 with a
    v_mask column appended per head (gives sum-of-exp for free in the PV matmul).
  attention, per (q-tile tau of 512, head-pair hp):
    S^T[k,q] = kwT.T @ qwT per 128-k block (two K=64 heads row-packed in the PE),
    P^T = exp(S^T/8) via ScalarE (psum->sbuf, bf16), causal zeroing of diagonal
    blocks via a DVE multiply with precomputed mask tiles, then
    oT[65, 512] += vw_aug.T @ P^T accumulated over k blocks (row 64 = sum of exp).
    oT is transposed back via PE transpose; rows are scaled by 1/sumexp on DVE.
v_mask is pre-applied to v on host (and to the ones column via vmask_t on device);
q_mask is applied to the returned output on host.  Masks are {0,1} so this is exact.
"""
import numpy as np
import ml_dtypes
from contextlib import ExitStack

import concourse.bass as bass
import concourse.tile as tile
from concourse import bacc, mybir
from concourse.bass_utils import run_bass_kernel_spmd
from concourse.masks import make_identity

F32 = mybir.dt.float32
BF16 = mybir.dt.bfloat16
BF16_NP = ml_dtypes.bfloat16

L = 2048          # sequence length
D = 1024          # d_model
COLS = 512        # projection columns per core (8 heads x 64)
NKSUB = L // 128  # 16 k-subtiles
NTAU = L // 512   # 4 q-tiles
NHP = 4           # head pairs per core


def _build_kernel(interleave=True, sps_bufs=2, pt_bufs=6, reps=1,
                  do_proj=True, do_attn=True, do_mm=True, exp_split=False,
                  pj_share=True, unified_psum=False):
    nc = bacc.Bacc("TRN2", target_bir_lowering=False, debug=False, num_devices=8)

    qTt = nc.dram_tensor("qTt", [512, 4096], BF16, kind="ExternalInput").ap()
    kTt = nc.dram_tensor("kTt", [512, 4096], BF16, kind="ExternalInput").ap()
    vTt = nc.dram_tensor("vTt", [512, 4096], BF16, kind="ExternalInput").ap()
    wq = nc.dram_tensor("wq", [128, 4096], BF16, kind="ExternalInput").ap()
    wk = nc.dram_tensor("wk", [128, 4096], BF16, kind="ExternalInput").ap()
    wv = nc.dram_tensor("wv", [128, 4096], BF16, kind="ExternalInput").ap()
    vmask_t = nc.dram_tensor("vmask_t", [128, NKSUB], F32, kind="ExternalInput").ap()
    out = nc.dram_tensor("out", [L, COLS], F32, kind="ExternalOutput").ap()

    with tile.TileContext(nc) as tc, ExitStack() as ctx:
        sb = ctx.enter_context(tc.tile_pool(name="sb", bufs=1))
        ps = ctx.enter_context(tc.tile_pool(name="ps", bufs=1, space="PSUM"))

        # ---- persistent SBUF tensors ----
        w_t = {t: sb.tile([128, 4096], BF16, tag="w", bufs=3, name=f"w{t}")
               for t in ("q", "k", "v")}
        w_loaded = set()

        def load_weights(tname):
            if tname in w_loaded:
                return
            w_loaded.add(tname)
            src = {"q": wq, "k": wk, "v": wv}[tname]
            # halves: d=0..3 matmuls can start after the first 512KB lands
            nc.sync.dma_start(w_t[tname][:, 0:2048], src[:, 0:2048])
            nc.sync.dma_start(w_t[tname][:, 2048:4096], src[:, 2048:4096])

        vmask_sb = sb.tile([128, NKSUB], F32, tag="vm")
        nc.sync.dma_start(vmask_sb[:], vmask_t[:])
        ident = sb.tile([128, 128], F32, tag="id")
        make_identity(nc, ident[:])

        # causal masks for the two diagonal k-batches (c = b - 2*tau in {0, 1}):
        # m_c[p, j, q] = 1 if q >= 256c + 128j + p else 0
        mask_c = []
        for c in range(2):
            m = sb.tile([128, 1024], BF16, tag="mask", bufs=2, name=f"mask{c}")
            nc.gpsimd.memset(m[:], 1.0)
            m3 = m[:].rearrange("p (j q) -> p j q", j=2)
            nc.gpsimd.affine_select(
                out=m3, in_=m3, compare_op=mybir.AluOpType.is_ge, fill=0.0,
                base=-256 * c, channel_multiplier=-1,
                pattern=[[-128, 2], [1, 512]])
            mask_c.append(m)

        qwT = [sb.tile([128, L], BF16, tag="qwT", bufs=NHP, name=f"qwT{hp}") for hp in range(NHP)]
        kwT = [sb.tile([128, L], BF16, tag="kwT", bufs=NHP, name=f"kwT{hp}") for hp in range(NHP)]
        vw_aug = [sb.tile([128, 8 * 65], BF16, tag="vwa", bufs=NKSUB, name=f"vwa{u}")
                  for u in range(NKSUB)]

        def proj_block(tname, lb):
            """Project one L-block of 512 for tensor tname in {q, k, v}."""
            load_weights(tname)
            src = {"q": qTt, "k": kTt, "v": vTt}[tname]
            act = sb.tile([128, 4096], BF16, tag="act", bufs=6, name=f"a{tname}{lb}")
            if tname == "k" and lb == 0:
                # split the very first load so d=0..3 matmuls start after 512KB
                nc.sync.dma_start(act[:, 0:2048],
                                  src[lb * 128:(lb + 1) * 128, 0:2048])
                nc.sync.dma_start(act[:, 2048:4096],
                                  src[lb * 128:(lb + 1) * 128, 2048:4096])
            else:
                nc.sync.dma_start(act[:], src[lb * 128:(lb + 1) * 128, :])
            if not do_mm:
                return
            wt = w_t[tname]
            pj_tag, pj_shape = (("sps", [128, 1024]) if pj_share
                                else ("pj", [128, 512]))
            if tname != "v":
                dst = qwT if tname == "q" else kwT
                for hp in range(NHP):
                    p = ps.tile(pj_shape, F32, tag=pj_tag,
                                bufs=sps_bufs if pj_share else 2,
                                name=f"pj{tname}{lb}{hp}")
                    p = p[:, 0:512]
                    for d in range(8):
                        nc.tensor.matmul(p[:],
                                         wt[:, d * 512 + hp * 128:d * 512 + (hp + 1) * 128],
                                         act[:, d * 512:(d + 1) * 512],
                                         start=(d == 0), stop=(d == 7),
                                         skip_group_check=True)
                    nc.vector.tensor_copy(dst[hp][:, lb * 512:(lb + 1) * 512], p[:])
            else:
                for ls in range(4):
                    u = lb * 4 + ls
                    p = ps.tile(pj_shape, F32, tag=pj_tag,
                                bufs=sps_bufs if pj_share else 2, name=f"pjv{u}")
                    p = p[:, 0:512]
                    for d in range(8):
                        nc.tensor.matmul(p[:],
                                         act[:, d * 512 + ls * 128:d * 512 + ls * 128 + 128],
                                         wt[:, d * 512:(d + 1) * 512],
                                         start=(d == 0), stop=(d == 7),
                                         skip_group_check=True)
                    v3d = vw_aug[u][:].rearrange("p (h c) -> p h c", h=8)
                    nc.vector.tensor_copy(v3d[:, :, 0:64],
                                          p[:].rearrange("p (h c) -> p h c", h=8))
                    nc.vector.tensor_copy(
                        v3d[:, :, 64:65].squeeze(2),
                        vmask_sb[:, u:u + 1].broadcast_to([128, 8]))

        oo_tiles = {}

        def attn_hp(tau, hp):
            """Attention for q-tile tau, one head pair."""
            if hp == 0:
                oo_tiles[tau] = sb.tile([128, 4 * COLS], F32, tag="oo", bufs=2,
                                        name=f"oo{tau}")
            oo = oo_tiles[tau]
            if True:
                otp = [ps.tile([65, 512], F32, tag="ot", bufs=2,
                               name=f"ot{tau}{hp}{x}") for x in range(2)]
                kmax = 4 * tau + 3  # last k-subtile (causal)
                for b in range(2 * (tau + 1)):  # batches of 2 k-subtiles
                    diag = b >= 2 * tau
                    # col0[j]: first unmasked q-column of block kappa=2b+j
                    # (q >= k requires q_col >= 128*(kappa - 4*tau); everything
                    # left of that is exactly zero after masking, so skip it)
                    col0 = [128 * max(0, 2 * b + j - 4 * tau) for j in range(2)]
                    for half in range(2):
                        h = hp * 2 + half
                        s = ps.tile([128, 1024], F32, tag="sps", bufs=sps_bufs,
                                    name=f"ss{tau}{hp}{b}{half}")
                        for j in range(2):
                            u = 2 * b + j
                            nc.tensor.matmul(
                                s[:, j * 512 + col0[j]:(j + 1) * 512],
                                kwT[hp][64 * half:64 * half + 64, u * 128:(u + 1) * 128],
                                qwT[hp][64 * half:64 * half + 64,
                                        tau * 512 + col0[j]:(tau + 1) * 512],
                                start=True, stop=True, skip_group_check=True,
                                tile_position=(64 * half, 0))
                        pt = sb.tile([128, 1024], BF16, tag="pT", bufs=pt_bufs,
                                     name=f"pt{tau}{hp}{b}{half}")
                        for j in range(2) if diag else (slice(None),):
                            if diag:
                                sl = slice(j * 512 + col0[j], (j + 1) * 512)
                            else:
                                sl = slice(None)
                            nc.scalar.activation(pt[:, sl], s[:, sl],
                                                 mybir.ActivationFunctionType.Exp,
                                                 scale=0.125)
                            if diag:  # causal zeroing within the block (DVE)
                                nc.vector.tensor_mul(pt[:, sl], pt[:, sl],
                                                     mask_c[b - 2 * tau][:, sl])
                            for jj in ((j,) if diag else (0, 1)):
                                u = 2 * b + jj
                                nc.tensor.matmul(
                                    otp[half][:, col0[jj]:512],
                                    vw_aug[u][:, h * 65:h * 65 + 65],
                                    pt[:, jj * 512 + col0[jj]:(jj + 1) * 512],
                                    start=(u == 0), stop=(u == kmax),
                                    skip_group_check=True)
                ot_sb = []
                for half in range(2):
                    o1 = sb.tile([65, 512], F32, tag="otsb", bufs=4,
                                 name=f"osb{tau}{hp}{half}")
                    nc.vector.tensor_copy(o1[:], otp[half][:])
                    ot_sb.append(o1)
                for qs in range(4):
                    if unified_psum:
                        otr = ps.tile([128, 1024], F32, tag="sps", bufs=sps_bufs,
                                      name=f"otr{tau}{hp}{qs}")[:, 0:130]
                    else:
                        otr = ps.tile([128, 130], F32, tag="pj", bufs=2,
                                      name=f"otr{tau}{hp}{qs}")
                    for half in range(2):
                        nc.tensor.transpose(
                            otr[:, 65 * half:65 * half + 65],
                            ot_sb[half][:, qs * 128:(qs + 1) * 128],
                            ident[0:65, 0:65])
                    rc = sb.tile([128, 2], F32, tag="rc", bufs=4,
                                 name=f"rc{tau}{hp}{qs}")
                    nc.vector.reciprocal(rc[:], otr[:, 64:130:65])
                    for half in range(2):
                        h = hp * 2 + half
                        nc.vector.tensor_scalar_mul(
                            oo[:, qs * COLS + h * 64:qs * COLS + (h + 1) * 64],
                            otr[:, 65 * half:65 * half + 64],
                            rc[:, half:half + 1])
            if hp == NHP - 1:
                # stores: one 256KB contiguous DMA per 128-row block
                for qs in range(4):
                    row = tau * 512 + qs * 128
                    nc.sync.dma_start(out[row:row + 128, :],
                                      oo[:, qs * COLS:(qs + 1) * COLS])

        def attn_tau(tau):
            for hp in range(NHP):
                attn_hp(tau, hp)

        if not do_proj and do_attn:
            # timing-only variant: initialize attention inputs so reads are legal
            for t in qwT + kwT:
                nc.gpsimd.memset(t[:], 0.0)
            for t in vw_aug:
                nc.gpsimd.memset(t[:], 1.0)
        for _rep in range(reps):
            w_loaded.clear()
            if interleave == "fine" and do_proj and do_attn:
                proj_block("k", 0)
                proj_block("v", 0)
                proj_block("q", 0)
                for tau in range(NTAU):
                    nxt = ([("k", tau + 1), ("v", tau + 1), ("q", tau + 1)]
                           if tau < NTAU - 1 else [])
                    for hp in range(NHP):
                        attn_hp(tau, hp)
                        if hp < len(nxt):
                            proj_block(*nxt[hp])
            elif interleave:
                for tau in range(NTAU):
                    if do_proj:
                        proj_block("k", tau)
                        proj_block("v", tau)
                        proj_block("q", tau)
                    if do_attn:
                        attn_tau(tau)
            else:
                if do_proj:
                    for tname in ("k", "v", "q"):
                        for lb in range(4):
                            proj_block(tname, lb)
                if do_attn:
                    for tau in range(NTAU):
                        attn_tau(tau)

    nc.compile()
    return nc


_NC_CACHE = None


def _get_nc():
    global _NC_CACHE
    if _NC_CACHE is None:
        _NC_CACHE = _build_kernel()
    return _NC_CACHE


def _tile_act(x):
    """[2048, 1024] fp32 -> [512, 4096] bf16 with [lb*128+p, d*512+c] layout."""
    t = x.reshape(4, 512, 8, 128).transpose(0, 3, 2, 1)  # [lb, p, d, c]
    return np.ascontiguousarray(t.reshape(512, 4096).astype(BF16_NP))


def _tile_w(w):
    """[1024, 512] fp32 -> [128, 4096] bf16 with [p, d*512+c] layout."""
    t = w.reshape(8, 128, 512).transpose(1, 0, 2)  # [p, d, c]
    return np.ascontiguousarray(t.reshape(128, 4096).astype(BF16_NP))


def make_in_maps(q, k, v, v_mask, q_mask, Wq, Wk, Wv):
    q = np.asarray(q, np.float32)
    k = np.asarray(k, np.float32)
    v = np.asarray(v, np.float32)
    v_mask = np.asarray(v_mask, np.float32)
    Wq = np.asarray(Wq, np.float32)
    Wk = np.asarray(Wk, np.float32)
    Wv = np.asarray(Wv, np.float32)
    in_maps = []
    for core in range(8):
        b, g = core // 2, core % 2
        cs = slice(g * COLS, (g + 1) * COLS)
        vp = v[b] * v_mask[b][:, None]
        in_maps.append({
            "qTt": _tile_act(q[b]),
            "kTt": _tile_act(k[b]),
            "vTt": _tile_act(vp),
            "wq": _tile_w(Wq[:, cs]),
            "wk": _tile_w(Wk[:, cs]),
            "wv": _tile_w(Wv[:, cs]),
            "vmask_t": np.ascontiguousarray(v_mask[b].reshape(NKSUB, 128).T),
        })
    return in_maps


def kernel(q, k, v, v_mask, q_mask, Wq, Wk, Wv):
    nc = _get_nc()
    in_maps = make_in_maps(q, k, v, v_mask, q_mask, Wq, Wk, Wv)
    res = run_bass_kernel_spmd(nc, in_maps, core_ids=list(range(8)))
    q_mask = np.asarray(q_mask, np.float32)
    out = np.empty((4, L, 2 * COLS), np.float32)
    for core in range(8):
        b, g = core // 2, core % 2
        out[b, :, g * COLS:(g + 1) * COLS] = res.results[core]["out"]
    out *= q_mask[:, :, None]
    return out

